# revision 29
# baseline (speedup 1.0000x reference)
"""Fused single-head attention with query-sum output, for 8 Trainium2 cores.

Reference computation (per batch b of 16):
    q = x @ Wq + bq ; k = x @ Wk + bk ; v = x @ Wv + bv        [S, D]
    energy = q @ k.T / sqrt(D)                                  [S, S]
    attn   = softmax(energy, axis=-1)
    out    = (attn @ v).sum(axis=0)                             [D]

Algebraic restructuring: out = colsum @ v_nobias + S * bv, where
colsum[k] = sum_q attn[q, k] = sum_q w[q] * E[q, k] with E = exp(energy)
and w[q] = 1 / sum_k E[q, k].  This replaces the O(S^2 D) attn @ v matmul
with an O(S^2) weighted column reduction plus a single matvec against v.
bk is dropped entirely: softmax is invariant to per-row shifts, and the
only bias term that varies along k is bq . k0 -- which is what you get by
biasing q alone.

fp8 (TRN e4m3, max 240) with DoubleRow double-pumped matmuls:
  - qT/kT are stored fp8; each 512-wide energy slice is ONE DoubleRow
    matmul contracting all 256 d-rows (2x PE throughput vs bf16).
  - E = exp(energy/sqrt(D) - 4*ln2) is written fp8 straight from the
    ACT engine.  The 2^-4 shift keeps max-logit exp (~e^7.1) under the
    fp8 Inf ceiling of 240 and cancels exactly in w = 1/Z because Z
    accumulates the same shifted values.
  - w is computed as 512/Z' (fp8-representable range) by feeding the
    reciprocal with Z'/512; the extra 512 is compensated for free by
    setting the colsum-transpose moving constant one_sb = 1/512.
  - colsum is ONE DoubleRow matmul per (tile-pair, 512-slice): the two
    tiles' E rows are interleaved in an [P, 2, S] fp8 tile and their two
    w columns in an [P, 2, 1] fp8 stationary.

Engine balance: ScalarE does ONLY the exp stream (+ batch-0 k evac while
no exps exist yet); all other PSUM evacuations go to the DVE.  The v
projection and batch-1 q/k projection are emitted as background units
interleaved between attention tiles so their PE/DVE work hides under the
ScalarE-paced exp stream.  The PE warmup is 16 back-to-back 256-wide
matmuls (~3.4us busy = one full HAM activity window) so the 2.4 GHz
clock gate opens before the first projection matmul.

Sharding: pure data-parallel over the batch dim -- 2 batches per core on
8 cores, full (tiny) weights replicated.  No collectives.
"""

import numpy as np
import ml_dtypes

import concourse.bass as bass
import concourse.mybir as mybir
import concourse.tile as tile
from concourse.bass import ts, ds
from concourse.bass_utils import run_bass_kernel_spmd

B, S, D = 16, 2048, 256
N_CORES = 8
BPC = B // N_CORES          # batches per core
P = 128
CC = D // P                 # contraction chunks over d (2)
DT = D // P                 # output-d tiles (2)
ST = S // P                 # 128-row tiles of the sequence (16)
NS = S // 512               # 512-wide slices of the sequence (4)
NP = ST // 2                # tile pairs (8)
F32 = mybir.dt.float32
BF16 = mybir.dt.bfloat16
FP8 = mybir.dt.float8e4
EXP = mybir.ActivationFunctionType.Exp
DR = mybir.MatmulPerfMode.DoubleRow
MULT = mybir.AluOpType.mult
ADD = mybir.AluOpType.add
INV_SQRT_D = 1.0 / np.sqrt(D)
E_SHIFT = -4.0 * float(np.log(2.0))  # exp output scaled by 2^-4, cancels in w
W_SCALE = 512.0                      # w stored as 512/Z', undone via one_sb
# Schraudolph bit-trick exp on the DVE: affine the logit into the bf16 bit
# pattern of 2^(x/ln2), write as int16, reinterpret as bf16.  Folds the
# 1/sqrt(D) logit scale and the 2^-4 shift (-4*128) into the constants.
SCH_SCALE = 128.0 / (float(np.log(2.0)) * 16.0)
SCH_BIAS = float(127 * 128 - 4 * 128)

_MAX_WAITS = 1  # this container's walrus rejects >1 sync wait per instruction


def _split_wide_waits(nc, max_waits=_MAX_WAITS):
    """walrus CoreV3 codegen here rejects instructions with more than one
    sync wait ("Too many sync wait commands").  Move excess waits onto
    freshly inserted same-engine NoOps placed immediately before the wide
    instruction (engine program order preserves semantics)."""
    n_split = 0
    for f in nc.m.functions:
        for blk in f.blocks:
            out = []
            changed = False
            for ins in blk.instructions:
                si = ins.sync_info
                if si is not None and len(si.on_wait) > max_waits:
                    waits = list(si.on_wait)
                    extra, keep = waits[:-max_waits], waits[-max_waits:]
                    for ci in range(0, len(extra), max_waits):
                        nop = mybir.InstNoOp(
                            name=f"I-waitfix-{nc.next_id()}", ins=[], outs=[]
                        )
                        nop.engine = ins.engine
                        nop.sync_info = mybir.SyncInfo(
                            on_wait=extra[ci : ci + max_waits], on_update=[]
                        )
                        out.append(nop)
                        n_split += 1
                    si.on_wait = keep
                    changed = True
                out.append(ins)
            if changed:
                blk.instructions = out
    return n_split


def build_attention_nc():
    nc = bass.Bass(trn_type="TRN2")

    xt = nc.dram_tensor("xt", [BPC, D, S], BF16, kind="ExternalInput")
    wq = nc.dram_tensor("wq", [D, D], BF16, kind="ExternalInput")
    wk = nc.dram_tensor("wk", [D, D], BF16, kind="ExternalInput")
    wv = nc.dram_tensor("wv", [D, D], BF16, kind="ExternalInput")
    bq = nc.dram_tensor("bq", [D], F32, kind="ExternalInput")
    y = nc.dram_tensor("y", [BPC, D], F32, kind="ExternalOutput")

    with tile.TileContext(nc) as tc:
        with (
            tc.tile_pool(name="singles", bufs=1) as singles,
            tc.tile_pool(name="xT_pool", bufs=2) as xT_pool,
            tc.tile_pool(name="qkv_pool", bufs=2) as qkv_pool,
            tc.tile_pool(name="e_pool", bufs=3) as e_pool,
            tc.tile_pool(name="small_pool", bufs=4) as small_pool,
            tc.tile_pool(name="out_pool", bufs=2) as out_pool,
            tc.tile_pool(name="eps_pool", bufs=2, space="PSUM") as eps_pool,
        ):
            # ---- HAM warmup: one full 4096-cycle activity window of
            # back-to-back matmuls while the initial DMAs are in flight,
            # so the PE clock gate is at 8/8 before real work arrives. ----
            ones_bf = singles.tile([P, P], BF16, tag="ones_bf")
            nc.vector.memset(ones_bf[:], 1.0)
            zeros_bf = singles.tile([P, P], BF16, tag="zeros_bf")
            nc.vector.memset(zeros_bf[:], 0.0)
            with tc.tile_pool(name="warm_ps", bufs=1, space="PSUM") as wp:
                wm_ps = wp.tile([P, 256], F32, name="wm_ps")
                for _ in range(16):
                    nc.tensor.matmul(
                        wm_ps[:],
                        ones_bf[:],
                        ones_bf[:, 0:1].to_broadcast((P, 256)),
                        start=True,
                        stop=True,
                    )

            # ---- weights / constants ----
            wq_sb = singles.tile([P, CC, D], BF16, tag="wq")
            wk_sb = singles.tile([P, CC, D], BF16, tag="wk")
            wv_sb = singles.tile([P, CC, D], BF16, tag="wv")
            bq_sb = singles.tile([P, DT], F32, tag="bq")
            nc.sync.dma_start(wk_sb[:], wk.rearrange("(c p) d -> p c d", p=P))
            nc.sync.dma_start(wq_sb[:], wq.rearrange("(c p) d -> p c d", p=P))
            one_sb = singles.tile([P, 1], BF16, tag="one")
            nc.vector.memset(one_sb[:], 1.0 / W_SCALE)
            eshift_sb = singles.tile([P, 1], F32, tag="eshift")
            nc.vector.memset(eshift_sb[:], E_SHIFT)
            # dummy exp so the ACT table set loads at t~0 instead of on the
            # critical path right before the first real exp (~1.3-2.7us)
            warm_act = singles.tile([P, 1], BF16, tag="warm_act")
            nc.scalar.activation(warm_act[:], eshift_sb[:], EXP)

            # ---- prefetch both batches' x (host already transposed) ----
            xTs = []
            for b in range(BPC):
                xT = xT_pool.tile([P, CC, S], BF16, tag="xT", name=f"xT{b}")
                xt_r = xt[b].rearrange("(c p) s -> p c s", p=P)
                for sh in range(2):
                    for c in range(CC):
                        nc.sync.dma_start(
                            xT[:, c, ts(sh, S // 2)], xt_r[:, c, ts(sh, S // 2)]
                        )
                xTs.append(xT)
                if b == 0:
                    nc.sync.dma_start(
                        bq_sb[:], bq.rearrange("(t p) -> p t", p=P)
                    )
                    nc.sync.dma_start(
                        wv_sb[:], wv.rearrange("(c p) d -> p c d", p=P)
                    )

            def qk_psum_unit(b, w_sb, outT, dt_, ns, pp, bias):
                """One q/k projection PSUM tile: 2 bf16 matmuls + 1 evac.
                bias None -> plain DVE copy; AP -> DVE bias-add; the string
                'scalar' -> ScalarE copy (batch-0 k, before exps exist).
                """
                ps = pp.tile([P, 512], F32, tag="qk", name="ps_qk")
                for cc in range(CC):
                    nc.tensor.matmul(
                        ps[:],
                        w_sb[:, cc, ts(dt_, P)],
                        xTs[b][:, cc, ts(ns, 512)],
                        start=(cc == 0),
                        stop=(cc == CC - 1),
                    )
                dst = outT[:, dt_, ts(ns, 512)]
                if bias is None:
                    nc.vector.tensor_copy(dst, ps[:])
                elif isinstance(bias, str):
                    nc.scalar.copy(dst, ps[:])
                else:
                    nc.vector.tensor_scalar(dst, ps[:], bias, None, op0=ADD)

            def v_psum_unit(b, v, st, pp):
                vps = pp.tile([P, 512], F32, tag="qk", name="ps_v")
                for cc in range(CC):
                    nc.tensor.matmul(
                        vps[:, :D],
                        xTs[b][:, cc, ts(st, P)],
                        wv_sb[:, cc, :],
                        start=(cc == 0),
                        stop=(cc == CC - 1),
                    )
                nc.vector.tensor_copy(v[:, st, :], vps[:, :D])

            def attention(b, qT, kT, cp, bg_units, per_tile, offload=()):
                """energy -> exp(+row-sum) -> w-weighted column-sum.

                Each 512-wide energy slice is one fp8 DoubleRow matmul
                (256-deep); exp goes straight to fp8 E with the 2^-4
                shift; colsum runs per tile-PAIR as fp8 DoubleRow with
                the pair's two w columns stationary.  bg_units closures
                (v / next-batch projections / prev-batch matvec) are
                drained between tiles to hide under the ScalarE-paced
                exp stream."""
                # 4 colsum slices packed 2 rows (partitions 0/32) x 2 halves:
                # SBUF matmul reads can only start at partitions {0,32,64}.
                colsum_sb = small_pool.tile([P, 1024], BF16, tag="colsum_sb",
                                            name=f"colsum_sb{b}")
                cs_ps = cp.tile([P, 512], F32, name="cs_ps")
                # open the accumulation group: zero the whole bank
                nc.tensor.matmul(
                    cs_ps[:], zeros_bf[:], ones_bf[:, 0:1].to_broadcast((P, 512)),
                    start=True, stop=False, skip_group_check=True,
                )
                Es, wbs = [], []

                def emit_energy(t):
                    sch = t in offload
                    if sch:
                        E = e_pool.tile([P, S], BF16, tag="Es", name="Es")
                        wb = small_pool.tile([P, 1], BF16, tag="wbb", name="wbb")
                    else:
                        E = e_pool.tile([P, S], FP8, tag="E", name="E")
                        wb = small_pool.tile([P, 1], FP8, tag="wb", name="wb")
                        z2 = small_pool.tile([P, 2], F32, tag="z2", name="z2")
                    zs = small_pool.tile([P, 1], F32, tag="zs", name="zs")
                    for h in range(2):
                        eps = eps_pool.tile([P, 1024], F32, tag="e", name="ps_e")
                        for n2 in range(2):
                            nc.tensor.matmul(
                                eps[:, ts(n2, 512)],
                                qT[:, 0:CC, ts(t, P)],
                                kT[:, 0:CC, ds(h * 1024 + n2 * 512, 512)],
                                start=True,
                                stop=True,
                                perf_mode=DR,
                            )
                        if sch:
                            nc.vector.tensor_scalar(
                                E[:, ts(h, 1024)].bitcast(mybir.dt.int16),
                                eps[:], SCH_SCALE, SCH_BIAS,
                                op0=MULT, op1=ADD,
                            )
                        else:
                            nc.scalar.activation(
                                E[:, ts(h, 1024)],
                                eps[:],
                                EXP,
                                bias=eshift_sb[:],
                                scale=INV_SQRT_D,
                                accum_out=z2[:, h : h + 1],
                            )
                    if sch:
                        # Z from a 4x-mode bf16 sweep over E (scaled by 1/512)
                        scr = e_pool.tile([P, S], BF16, tag="schscr",
                                          name="schscr")
                        nc.vector.tensor_scalar(
                            scr[:], E[:], 1.0 / W_SCALE, 0.0,
                            op0=MULT, op1=ADD, accum_out=zs[:],
                        )
                    else:
                        # zs = (z0 + z1)/512 in one fused op
                        zsc = small_pool.tile([P, 2], F32, tag="zsc",
                                              name="zsc")
                        nc.vector.tensor_scalar(
                            zsc[:], z2[:], 1.0 / W_SCALE, 0.0,
                            op0=MULT, op1=ADD, accum_out=zs[:],
                        )
                    with nc.allow_low_precision(
                        reason="w->fp8/bf16 is the point: colsum averages it"
                    ):
                        nc.vector.reciprocal(wb[:], zs[:])
                    Es.append(E)
                    wbs.append(wb)

                def emit_colsum(t):
                    last = t == ST - 1
                    for ns in range(NS):
                        nc.tensor.matmul(
                            cs_ps[32 * ns : 32 * ns + 1, :],
                            wbs[t][:],
                            Es[t][:, ts(ns, 512)],
                            start=False,
                            stop=last and ns == NS - 1,
                            tile_position=(0, 32 * ns),
                            skip_group_check=True,
                        )

                nu = 0
                for t in range(ST):
                    emit_energy(t)
                    if t >= 2:
                        emit_colsum(t - 2)
                    take = min(per_tile, len(bg_units) - nu)
                    for _ in range(take):
                        bg_units[nu]()
                        nu += 1
                emit_colsum(ST - 2)
                emit_colsum(ST - 1)
                while nu < len(bg_units):
                    bg_units[nu]()
                    nu += 1
                for ns in range(NS):
                    r = 32 * (ns // 2)
                    dst = colsum_sb[r : r + 1, ts(ns % 2, 512)]
                    src = cs_ps[32 * ns : 32 * ns + 1, :]
                    # last batch: ScalarE is idle once its exps are done, so
                    # split the 4 serial evacs across both engines
                    if b == BPC - 1 and ns % 2 == 0:
                        nc.scalar.copy(dst, src)
                    else:
                        nc.vector.tensor_copy(dst, src)
                return colsum_sb

            def final_units(b, colsum_sb, v, fp):
                """Final matvec as 5 background units.  colT holds the
                512/W_SCALE compensation via one_sb."""
                fin_ps = fp.tile([P, 16 + D], F32, name=f"fin_ps{b}")
                colT_ps = fin_ps[:, 0:ST]
                out_ps = fin_ps[0:1, ST : ST + D]
                colT = small_pool.tile([P, ST], BF16, tag="colT",
                                       name=f"colT{b}")
                y_sb = out_pool.tile([1, D], F32, tag="y_sb", name=f"y_sb{b}")

                def transpose_half(h0):
                    for t in range(h0, h0 + ST // 2):
                        ns = t // 4
                        r = 32 * (ns // 2)
                        cb = 512 * (ns % 2) + P * (t % 4)
                        nc.tensor.matmul(
                            colT_ps[:, t : t + 1],
                            colsum_sb[r : r + 1, ds(cb, P)],
                            one_sb[r : r + 1, 0:1],
                            start=(t == 0),
                            stop=(t == ST - 1),
                        )

                def evac():
                    nc.vector.tensor_copy(colT[:], colT_ps[:])

                def matvec_half(h0):
                    for t in range(h0, h0 + ST // 2):
                        nc.tensor.matmul(
                            out_ps[:],
                            colT[:, t : t + 1],
                            v[:, t, :],
                            start=(t == 0),
                            stop=(t == ST - 1),
                        )

                def out_dma():
                    nc.vector.tensor_copy(y_sb[:], out_ps[:])
                    nc.sync.dma_start(y[b : b + 1, :], y_sb[:])

                return [
                    lambda: transpose_half(0),
                    lambda: transpose_half(ST // 2),
                    evac,
                    lambda: matvec_half(0),
                    lambda: matvec_half(ST // 2),
                    out_dma,
                ]

            # ---- batch 0 q/k projection, directly emitted ----
            # k evacs on ScalarE (idle until the first exp), q on DVE.
            q0T = qkv_pool.tile([P, DT, S], FP8, tag="qT", name="qT0")
            k0T = qkv_pool.tile([P, DT, S], FP8, tag="kT", name="kT0")
            q1T = qkv_pool.tile([P, DT, S], FP8, tag="qT", name="qT1")
            k1T = qkv_pool.tile([P, DT, S], FP8, tag="kT", name="kT1")
            v0 = qkv_pool.tile([P, ST, D], BF16, tag="v", name="v0")
            v1 = qkv_pool.tile([P, ST, D], BF16, tag="v", name="v1")

            with tc.tile_pool(name="proj_ps_0", bufs=2, space="PSUM") as pp0:
                # k0 evacs alternate ScalarE/DVE (both idle before the first
                # exp) so the full kT -- which gates the first energy tile --
                # lands as early as possible; then q's first chunk.
                for ns in range(NS):
                    for dt_ in range(DT):
                        eng = "scalar" if (2 * ns + dt_) % 2 == 0 else None
                        qk_psum_unit(0, wk_sb, k0T, dt_, ns, pp0, eng)
                for dt_ in range(DT):
                    qk_psum_unit(0, wq_sb, q0T, dt_, 0, pp0,
                                 bq_sb[:, dt_ : dt_ + 1])

            with tc.tile_pool(name="bg_ps", bufs=2, space="PSUM") as ppbg:
                # background work drained inside attention(0): rest of the q0
                # projection, batch-0 v projection, batch-1 k/q projection.
                units0 = [
                    (lambda ns=ns, dt_=dt_: qk_psum_unit(
                        0, wq_sb, q0T, dt_, ns, ppbg,
                        bq_sb[:, dt_ : dt_ + 1]))
                    for ns in range(1, NS)
                    for dt_ in range(DT)
                ]
                units0 += [
                    (lambda st=st: v_psum_unit(0, v0, st, ppbg))
                    for st in range(ST)
                ]
                for ns in range(NS):
                    for dt_ in range(DT):
                        units0.append(
                            lambda ns=ns, dt_=dt_: qk_psum_unit(
                                1, wk_sb, k1T, dt_, ns, ppbg, None
                            )
                        )
                for ns in range(NS):
                    for dt_ in range(DT):
                        units0.append(
                            lambda ns=ns, dt_=dt_: qk_psum_unit(
                                1, wq_sb, q1T, dt_, ns, ppbg,
                                bq_sb[:, dt_ : dt_ + 1],
                            )
                        )
                with tc.tile_pool(name="cs_ps_0", bufs=1, space="PSUM") as cp0:
                    cs0 = attention(0, q0T, k0T, cp0, units0, per_tile=3,
                                    offload=(10, 12, 14))
                # attention(1): drain batch-1 v projection + batch-0 final
                # matvec between its tiles.
                with tc.tile_pool(name="fin_ps_0", bufs=1, space="PSUM") as fp0:
                    units1 = [
                        (lambda st=st: v_psum_unit(1, v1, st, ppbg))
                        for st in range(ST)
                    ] + final_units(0, cs0, v0, fp0)
                    with tc.tile_pool(name="cs_ps_1", bufs=1,
                                      space="PSUM") as cp1:
                        cs1 = attention(1, q1T, k1T, cp1, units1, per_tile=2,
                                        offload=(5, 8, 11, 14))
                    with tc.tile_pool(name="fin_ps_1", bufs=1,
                                      space="PSUM") as fp1:
                        for u in final_units(1, cs1, v1, fp1):
                            u()

    _split_wide_waits(nc)
    return nc


_NC_CACHE = None


def _get_nc():
    global _NC_CACHE
    if _NC_CACHE is None:
        _NC_CACHE = build_attention_nc()
    return _NC_CACHE


def kernel(x, Wq, bq, Wk, bk, Wv, bv, _return_raw=False, _trace=False):
    x = np.asarray(x, dtype=np.float32)
    # pre-transpose on host: device wants the contraction dim on partitions
    xt_bf = np.ascontiguousarray(x.transpose(0, 2, 1)).astype(ml_dtypes.bfloat16)
    wq_bf = np.asarray(Wq, dtype=np.float32).astype(ml_dtypes.bfloat16)
    wk_bf = np.asarray(Wk, dtype=np.float32).astype(ml_dtypes.bfloat16)
    wv_bf = np.asarray(Wv, dtype=np.float32).astype(ml_dtypes.bfloat16)
    bq32 = np.ascontiguousarray(np.asarray(bq, dtype=np.float32))

    nc = _get_nc()
    in_maps = [
        {
            "xt": np.ascontiguousarray(xt_bf[i * BPC : (i + 1) * BPC]),
            "wq": wq_bf,
            "wk": wk_bf,
            "wv": wv_bf,
            "bq": bq32,
        }
        for i in range(N_CORES)
    ]
    res = run_bass_kernel_spmd(
        nc, in_maps, core_ids=list(range(N_CORES)), trace=_trace
    )
    out = np.concatenate([res.results[i]["y"] for i in range(N_CORES)], axis=0)
    out = out + S * np.asarray(bv, dtype=np.float32)[None, :]
    out = out.astype(np.float32)
    if _return_raw:
        return out, res
    return out


# revision 35
# speedup vs baseline: 1.0608x; 1.0608x over previous
"""Fused single-head attention with query-sum output, for 8 Trainium2 cores.

Reference computation (per batch b of 16):
    q = x @ Wq + bq ; k = x @ Wk + bk ; v = x @ Wv + bv        [S, D]
    energy = q @ k.T / sqrt(D)                                  [S, S]
    attn   = softmax(energy, axis=-1)
    out    = (attn @ v).sum(axis=0)                             [D]

Algebraic restructuring: out = colsum @ v_nobias + S * bv, where
colsum[k] = sum_q attn[q, k] = sum_q w[q] * E[q, k] with E = exp(energy)
and w[q] = 1 / sum_k E[q, k].  This replaces the O(S^2 D) attn @ v matmul
with an O(S^2) weighted column reduction plus a single matvec against v.
bk is dropped entirely: softmax is invariant to per-row shifts, and the
only bias term that varies along k is bq . k0 -- which is what you get by
biasing q alone.

fp8 (TRN e4m3, max 240) with DoubleRow double-pumped matmuls:
  - qT/kT are stored fp8; each 512-wide energy slice is ONE DoubleRow
    matmul contracting all 256 d-rows (2x PE throughput vs bf16).
  - E = exp(energy/sqrt(D) - 4*ln2) is written fp8 straight from the
    ACT engine.  The 2^-4 shift keeps max-logit exp (~e^7.1) under the
    fp8 Inf ceiling of 240 and cancels exactly in w = 1/Z because Z
    accumulates the same shifted values.
  - w is computed as 512/Z' (fp8-representable range) by feeding the
    reciprocal with Z'/512; the extra 512 is compensated for free by
    setting the colsum-transpose moving constant one_sb = 1/512.
  - colsum is ONE DoubleRow matmul per (tile-pair, 512-slice): the two
    tiles' E rows are interleaved in an [P, 2, S] fp8 tile and their two
    w columns in an [P, 2, 1] fp8 stationary.

Engine balance: ScalarE does ONLY the exp stream (+ batch-0 k evac while
no exps exist yet); all other PSUM evacuations go to the DVE.  The v
projection and batch-1 q/k projection are emitted as background units
interleaved between attention tiles so their PE/DVE work hides under the
ScalarE-paced exp stream.  The PE warmup is 16 back-to-back 256-wide
matmuls (~3.4us busy = one full HAM activity window) so the 2.4 GHz
clock gate opens before the first projection matmul.

Sharding: pure data-parallel over the batch dim -- 2 batches per core on
8 cores, full (tiny) weights replicated.  No collectives.
"""

import numpy as np
import ml_dtypes

import concourse.bass as bass
import concourse.mybir as mybir
import concourse.tile as tile
from concourse.bass import ts, ds
from concourse.bass_utils import run_bass_kernel_spmd

B, S, D = 16, 2048, 256
N_CORES = 8
BPC = B // N_CORES          # batches per core
P = 128
CC = D // P                 # contraction chunks over d (2)
DT = D // P                 # output-d tiles (2)
ST = S // P                 # 128-row tiles of the sequence (16)
NS = S // 512               # 512-wide slices of the sequence (4)
NP = ST // 2                # tile pairs (8)
F32 = mybir.dt.float32
BF16 = mybir.dt.bfloat16
FP8 = mybir.dt.float8e4
EXP = mybir.ActivationFunctionType.Exp
DR = mybir.MatmulPerfMode.DoubleRow
MULT = mybir.AluOpType.mult
ADD = mybir.AluOpType.add
INV_SQRT_D = 1.0 / np.sqrt(D)
E_SHIFT = -4.0 * float(np.log(2.0))  # exp output scaled by 2^-4, cancels in w
W_SCALE = 512.0                      # w stored as 512/Z', undone via one_sb
# Schraudolph bit-trick exp on the DVE: affine the logit into the bf16 bit
# pattern of 2^(x/ln2), write as int16, reinterpret as bf16.  Folds the
# 1/sqrt(D) logit scale and the 2^-4 shift (-4*128) into the constants.
SCH_SCALE = 128.0 / (float(np.log(2.0)) * 16.0)
SCH_BIAS = float(127 * 128 - 4 * 128)

_MAX_WAITS = 1  # this container's walrus rejects >1 sync wait per instruction


def _split_wide_waits(nc, max_waits=_MAX_WAITS):
    """walrus CoreV3 codegen here rejects instructions with more than one
    sync wait ("Too many sync wait commands").  Move excess waits onto
    freshly inserted same-engine NoOps placed immediately before the wide
    instruction (engine program order preserves semantics)."""
    n_split = 0
    for f in nc.m.functions:
        for blk in f.blocks:
            out = []
            changed = False
            for ins in blk.instructions:
                si = ins.sync_info
                if si is not None and len(si.on_wait) > max_waits:
                    waits = list(si.on_wait)
                    extra, keep = waits[:-max_waits], waits[-max_waits:]
                    for ci in range(0, len(extra), max_waits):
                        nop = mybir.InstNoOp(
                            name=f"I-waitfix-{nc.next_id()}", ins=[], outs=[]
                        )
                        nop.engine = ins.engine
                        nop.sync_info = mybir.SyncInfo(
                            on_wait=extra[ci : ci + max_waits], on_update=[]
                        )
                        out.append(nop)
                        n_split += 1
                    si.on_wait = keep
                    changed = True
                out.append(ins)
            if changed:
                blk.instructions = out
    return n_split


def build_attention_nc():
    nc = bass.Bass(trn_type="TRN2")

    xt = nc.dram_tensor("xt", [BPC, D, S], BF16, kind="ExternalInput")
    wq = nc.dram_tensor("wq", [D, D], BF16, kind="ExternalInput")
    wk = nc.dram_tensor("wk", [D, D], BF16, kind="ExternalInput")
    wv = nc.dram_tensor("wv", [D, D], BF16, kind="ExternalInput")
    bq = nc.dram_tensor("bq", [D], F32, kind="ExternalInput")
    y = nc.dram_tensor("y", [BPC, D], F32, kind="ExternalOutput")

    with tile.TileContext(nc) as tc:
        with (
            tc.tile_pool(name="singles", bufs=1) as singles,
            tc.tile_pool(name="xT_pool", bufs=2) as xT_pool,
            tc.tile_pool(name="qkv_pool", bufs=2) as qkv_pool,
            tc.tile_pool(name="e_pool", bufs=3) as e_pool,
            tc.tile_pool(name="small_pool", bufs=4) as small_pool,
            tc.tile_pool(name="out_pool", bufs=2) as out_pool,
            tc.tile_pool(name="eps_pool", bufs=2, space="PSUM") as eps_pool,
        ):
            # ---- HAM warmup: one full 4096-cycle activity window of
            # back-to-back matmuls while the initial DMAs are in flight,
            # so the PE clock gate is at 8/8 before real work arrives. ----
            ones_bf = singles.tile([P, P], BF16, tag="ones_bf")
            nc.vector.memset(ones_bf[:], 1.0)
            zeros_bf = singles.tile([P, P], BF16, tag="zeros_bf")
            nc.vector.memset(zeros_bf[:], 0.0)
            with tc.tile_pool(name="warm_ps", bufs=1, space="PSUM") as wp:
                wm_ps = wp.tile([P, 256], F32, name="wm_ps")
                for _ in range(16):
                    nc.tensor.matmul(
                        wm_ps[:],
                        ones_bf[:],
                        ones_bf[:, 0:1].to_broadcast((P, 256)),
                        start=True,
                        stop=True,
                    )

            # ---- weights / constants ----
            wq_sb = singles.tile([P, CC, D], BF16, tag="wq")
            wk_sb = singles.tile([P, CC, D], BF16, tag="wk")
            wv_sb = singles.tile([P, CC, D], BF16, tag="wv")
            bq_sb = singles.tile([P, DT], F32, tag="bq")
            nc.sync.dma_start(wk_sb[:], wk.rearrange("(c p) d -> p c d", p=P))
            nc.sync.dma_start(wq_sb[:], wq.rearrange("(c p) d -> p c d", p=P))
            one_sb = singles.tile([P, 1], BF16, tag="one")
            nc.vector.memset(one_sb[:], 1.0 / W_SCALE)
            eshift_sb = singles.tile([P, 1], F32, tag="eshift")
            nc.vector.memset(eshift_sb[:], E_SHIFT)
            # dummy exp so the ACT table set loads at t~0 instead of on the
            # critical path right before the first real exp (~1.3-2.7us)
            warm_act = singles.tile([P, 1], BF16, tag="warm_act")
            nc.scalar.activation(warm_act[:], eshift_sb[:], EXP)

            # ---- prefetch both batches' x (host already transposed) ----
            xTs = []
            for b in range(BPC):
                xT = xT_pool.tile([P, CC, S], BF16, tag="xT", name=f"xT{b}")
                xt_r = xt[b].rearrange("(c p) s -> p c s", p=P)
                for sh in range(2):
                    for c in range(CC):
                        nc.sync.dma_start(
                            xT[:, c, ts(sh, S // 2)], xt_r[:, c, ts(sh, S // 2)]
                        )
                xTs.append(xT)
                if b == 0:
                    nc.sync.dma_start(
                        bq_sb[:], bq.rearrange("(t p) -> p t", p=P)
                    )
                    nc.sync.dma_start(
                        wv_sb[:], wv.rearrange("(c p) d -> p c d", p=P)
                    )

            def qk_psum_unit(b, w_sb, outT, dt_, ns, pp, bias):
                """One q/k projection PSUM tile: 2 bf16 matmuls + 1 evac.
                bias None -> plain DVE copy; AP -> DVE bias-add; the string
                'scalar' -> ScalarE copy (batch-0 k, before exps exist).
                """
                ps = pp.tile([P, 512], F32, tag="qk", name="ps_qk")
                for cc in range(CC):
                    nc.tensor.matmul(
                        ps[:],
                        w_sb[:, cc, ts(dt_, P)],
                        xTs[b][:, cc, ts(ns, 512)],
                        start=(cc == 0),
                        stop=(cc == CC - 1),
                    )
                dst = outT[:, dt_, ts(ns, 512)]
                if bias is None:
                    nc.vector.tensor_copy(dst, ps[:])
                elif isinstance(bias, str):
                    nc.scalar.copy(dst, ps[:])
                else:
                    nc.vector.tensor_scalar(dst, ps[:], bias, None, op0=ADD)

            def v_psum_unit(b, v, st, pp):
                vps = pp.tile([P, 512], F32, tag="qk", name="ps_v")
                for cc in range(CC):
                    nc.tensor.matmul(
                        vps[:, :D],
                        xTs[b][:, cc, ts(st, P)],
                        wv_sb[:, cc, :],
                        start=(cc == 0),
                        stop=(cc == CC - 1),
                    )
                nc.vector.tensor_copy(v[:, st, :], vps[:, :D])

            def attention(b, qT, kT, cp, bg_units, per_tile, offload=()):
                """energy -> exp(+row-sum) -> w-weighted column-sum.

                Each 512-wide energy slice is one fp8 DoubleRow matmul
                (256-deep); exp goes straight to fp8 E with the 2^-4
                shift; colsum runs per tile-PAIR as fp8 DoubleRow with
                the pair's two w columns stationary.  bg_units closures
                (v / next-batch projections / prev-batch matvec) are
                drained between tiles to hide under the ScalarE-paced
                exp stream."""
                # 4 colsum slices packed 2 rows (partitions 0/32) x 2 halves:
                # SBUF matmul reads can only start at partitions {0,32,64}.
                colsum_sb = small_pool.tile([P, 1024], BF16, tag="colsum_sb",
                                            name=f"colsum_sb{b}")
                cs_ps = cp.tile([P, 512], F32, name="cs_ps")
                # open the accumulation group: zero the whole bank
                nc.tensor.matmul(
                    cs_ps[:], zeros_bf[:], ones_bf[:, 0:1].to_broadcast((P, 512)),
                    start=True, stop=False, skip_group_check=True,
                )
                Es, wbs = [], []

                def emit_energy(t):
                    sch = t in offload
                    if sch:
                        E = e_pool.tile([P, S], BF16, tag="Es", name="Es")
                        wb = small_pool.tile([P, 1], BF16, tag="wbb", name="wbb")
                    else:
                        E = e_pool.tile([P, S], FP8, tag="E", name="E")
                        wb = small_pool.tile([P, 1], FP8, tag="wb", name="wb")
                        z2 = small_pool.tile([P, 2], F32, tag="z2", name="z2")
                    zs = small_pool.tile([P, 1], F32, tag="zs", name="zs")
                    for h in range(2):
                        eps = eps_pool.tile([P, 1024], F32, tag="e", name="ps_e")
                        for n2 in range(2):
                            nc.tensor.matmul(
                                eps[:, ts(n2, 512)],
                                qT[:, 0:CC, ts(t, P)],
                                kT[:, 0:CC, ds(h * 1024 + n2 * 512, 512)],
                                start=True,
                                stop=True,
                                perf_mode=DR,
                            )
                        if sch:
                            nc.vector.tensor_scalar(
                                E[:, ts(h, 1024)].bitcast(mybir.dt.int16),
                                eps[:], SCH_SCALE, SCH_BIAS,
                                op0=MULT, op1=ADD,
                            )
                        else:
                            nc.scalar.activation(
                                E[:, ts(h, 1024)],
                                eps[:],
                                EXP,
                                bias=eshift_sb[:],
                                scale=INV_SQRT_D,
                                accum_out=z2[:, h : h + 1],
                            )
                    if sch:
                        # Z on the (otherwise idle) GpSimd: bf16 SBUF sweep
                        # over E scaled by 1/512, accumulated per partition
                        scr = e_pool.tile([P, S], BF16, tag="schscr",
                                          name="schscr")
                        nc.vector.tensor_scalar(
                            scr[:], E[:], 1.0 / W_SCALE, 0.0,
                            op0=MULT, op1=ADD, accum_out=zs[:],
                        )
                    else:
                        # zs = (z0 + z1)/512 in one fused op
                        zsc = small_pool.tile([P, 2], F32, tag="zsc",
                                              name="zsc")
                        nc.vector.tensor_scalar(
                            zsc[:], z2[:], 1.0 / W_SCALE, 0.0,
                            op0=MULT, op1=ADD, accum_out=zs[:],
                        )
                    with nc.allow_low_precision(
                        reason="w->fp8/bf16 is the point: colsum averages it"
                    ):
                        nc.vector.reciprocal(wb[:], zs[:])
                    Es.append(E)
                    wbs.append(wb)

                def emit_colsum(t):
                    last = t == ST - 1
                    for ns in range(NS):
                        nc.tensor.matmul(
                            cs_ps[32 * ns : 32 * ns + 1, :],
                            wbs[t][:],
                            Es[t][:, ts(ns, 512)],
                            start=False,
                            stop=last and ns == NS - 1,
                            tile_position=(0, 32 * ns),
                            skip_group_check=True,
                        )

                nu = 0
                for t in range(ST):
                    emit_energy(t)
                    if t >= 2:
                        emit_colsum(t - 2)
                    take = min(per_tile, len(bg_units) - nu)
                    for _ in range(take):
                        bg_units[nu]()
                        nu += 1
                emit_colsum(ST - 2)
                emit_colsum(ST - 1)
                while nu < len(bg_units):
                    bg_units[nu]()
                    nu += 1
                for ns in range(NS):
                    r = 32 * (ns // 2)
                    dst = colsum_sb[r : r + 1, ts(ns % 2, 512)]
                    src = cs_ps[32 * ns : 32 * ns + 1, :]
                    # last batch: ScalarE is idle once its exps are done, so
                    # split the 4 serial evacs across both engines
                    if b == BPC - 1 and ns % 2 == 0:
                        nc.scalar.copy(dst, src)
                    else:
                        nc.vector.tensor_copy(dst, src)
                return colsum_sb

            def final_units(b, colsum_sb, v, fp):
                """Final matvec as 5 background units.  colT holds the
                512/W_SCALE compensation via one_sb."""
                fin_ps = fp.tile([P, 16 + D], F32, name=f"fin_ps{b}")
                colT_ps = fin_ps[:, 0:ST]
                out_ps = fin_ps[0:1, ST : ST + D]
                colT = small_pool.tile([P, ST], BF16, tag="colT",
                                       name=f"colT{b}")
                y_sb = out_pool.tile([1, D], F32, tag="y_sb", name=f"y_sb{b}")

                def transpose_half(h0):
                    for t in range(h0, h0 + ST // 2):
                        ns = t // 4
                        r = 32 * (ns // 2)
                        cb = 512 * (ns % 2) + P * (t % 4)
                        nc.tensor.matmul(
                            colT_ps[:, t : t + 1],
                            colsum_sb[r : r + 1, ds(cb, P)],
                            one_sb[r : r + 1, 0:1],
                            start=(t == 0),
                            stop=(t == ST - 1),
                        )

                def evac():
                    nc.vector.tensor_copy(colT[:], colT_ps[:])

                def matvec_half(h0):
                    for t in range(h0, h0 + ST // 2):
                        nc.tensor.matmul(
                            out_ps[:],
                            colT[:, t : t + 1],
                            v[:, t, :],
                            start=(t == 0),
                            stop=(t == ST - 1),
                        )

                def out_dma():
                    nc.vector.tensor_copy(y_sb[:], out_ps[:])
                    nc.sync.dma_start(y[b : b + 1, :], y_sb[:])

                return [
                    lambda: transpose_half(0),
                    lambda: transpose_half(ST // 2),
                    evac,
                    lambda: matvec_half(0),
                    lambda: matvec_half(ST // 2),
                    out_dma,
                ]

            # ---- batch 0 q/k projection, directly emitted ----
            # k evacs on ScalarE (idle until the first exp), q on DVE.
            q0T = qkv_pool.tile([P, DT, S], FP8, tag="qT", name="qT0")
            k0T = qkv_pool.tile([P, DT, S], FP8, tag="kT", name="kT0")
            q1T = qkv_pool.tile([P, DT, S], FP8, tag="qT", name="qT1")
            k1T = qkv_pool.tile([P, DT, S], FP8, tag="kT", name="kT1")
            v0 = qkv_pool.tile([P, ST, D], BF16, tag="v", name="v0")
            v1 = qkv_pool.tile([P, ST, D], BF16, tag="v", name="v1")

            with tc.tile_pool(name="proj_ps_0", bufs=2, space="PSUM") as pp0:
                # k0 evacs alternate ScalarE/DVE (both idle before the first
                # exp) so the full kT -- which gates the first energy tile --
                # lands as early as possible; then q's first chunk.
                for ns in range(NS):
                    for dt_ in range(DT):
                        eng = "scalar" if (2 * ns + dt_) % 2 == 0 else None
                        qk_psum_unit(0, wk_sb, k0T, dt_, ns, pp0, eng)
                for dt_ in range(DT):
                    qk_psum_unit(0, wq_sb, q0T, dt_, 0, pp0,
                                 bq_sb[:, dt_ : dt_ + 1])

            with tc.tile_pool(name="bg_ps", bufs=2, space="PSUM") as ppbg:
                # background work drained inside attention(0): rest of the q0
                # projection, batch-0 v projection, batch-1 k/q projection.
                units0 = [
                    (lambda ns=ns, dt_=dt_: qk_psum_unit(
                        0, wq_sb, q0T, dt_, ns, ppbg,
                        bq_sb[:, dt_ : dt_ + 1]))
                    for ns in range(1, NS)
                    for dt_ in range(DT)
                ]
                units0 += [
                    (lambda st=st: v_psum_unit(0, v0, st, ppbg))
                    for st in range(ST)
                ]
                for ns in range(NS):
                    for dt_ in range(DT):
                        units0.append(
                            lambda ns=ns, dt_=dt_: qk_psum_unit(
                                1, wk_sb, k1T, dt_, ns, ppbg, None
                            )
                        )
                for ns in range(NS):
                    for dt_ in range(DT):
                        units0.append(
                            lambda ns=ns, dt_=dt_: qk_psum_unit(
                                1, wq_sb, q1T, dt_, ns, ppbg,
                                bq_sb[:, dt_ : dt_ + 1],
                            )
                        )
                with tc.tile_pool(name="cs_ps_0", bufs=1, space="PSUM") as cp0:
                    cs0 = attention(0, q0T, k0T, cp0, units0, per_tile=3)
                # attention(1): drain batch-1 v projection + batch-0 final
                # matvec between its tiles.
                with tc.tile_pool(name="fin_ps_0", bufs=1, space="PSUM") as fp0:
                    units1 = [
                        (lambda st=st: v_psum_unit(1, v1, st, ppbg))
                        for st in range(ST)
                    ] + final_units(0, cs0, v0, fp0)
                    with tc.tile_pool(name="cs_ps_1", bufs=1,
                                      space="PSUM") as cp1:
                        cs1 = attention(1, q1T, k1T, cp1, units1, per_tile=2)
                    with tc.tile_pool(name="fin_ps_1", bufs=1,
                                      space="PSUM") as fp1:
                        for u in final_units(1, cs1, v1, fp1):
                            u()

    _split_wide_waits(nc)
    return nc


_NC_CACHE = None


def _get_nc():
    global _NC_CACHE
    if _NC_CACHE is None:
        _NC_CACHE = build_attention_nc()
    return _NC_CACHE


def kernel(x, Wq, bq, Wk, bk, Wv, bv, _return_raw=False, _trace=False):
    x = np.asarray(x, dtype=np.float32)
    # pre-transpose on host: device wants the contraction dim on partitions
    xt_bf = np.ascontiguousarray(x.transpose(0, 2, 1)).astype(ml_dtypes.bfloat16)
    wq_bf = np.asarray(Wq, dtype=np.float32).astype(ml_dtypes.bfloat16)
    wk_bf = np.asarray(Wk, dtype=np.float32).astype(ml_dtypes.bfloat16)
    wv_bf = np.asarray(Wv, dtype=np.float32).astype(ml_dtypes.bfloat16)
    bq32 = np.ascontiguousarray(np.asarray(bq, dtype=np.float32))

    nc = _get_nc()
    in_maps = [
        {
            "xt": np.ascontiguousarray(xt_bf[i * BPC : (i + 1) * BPC]),
            "wq": wq_bf,
            "wk": wk_bf,
            "wv": wv_bf,
            "bq": bq32,
        }
        for i in range(N_CORES)
    ]
    res = run_bass_kernel_spmd(
        nc, in_maps, core_ids=list(range(N_CORES)), trace=_trace
    )
    out = np.concatenate([res.results[i]["y"] for i in range(N_CORES)], axis=0)
    out = out + S * np.asarray(bv, dtype=np.float32)[None, :]
    out = out.astype(np.float32)
    if _return_raw:
        return out, res
    return out
